# revision 2
# baseline (speedup 1.0000x reference)
"""Linear-attention (relu, rmsnorm-qk) Trainium2 Bass kernel, 8 NeuronCores.

Sharding: each core owns 1/4 of the tokens of TWO batch elements:
  cores 0-3 -> batches 0 (group g=0) and 1 (g=1)
  cores 4-7 -> batches 2 (g=0) and 3 (g=1)
Within a batch, core q (= core_id % 4) owns tokens [1024*q, 1024*(q+1)).

Math restructure vs the straightforward version:
  - q's rmsnorm is dropped entirely: out = relu(q)kv / (relu(q)ksum + eps)
    is invariant to positive per-token scaling of q (the eps shift is
    O(1e-12) relative).  qn_w is folded into W_q columns on the host
    (exact: relu(c*qn*q) = c*relu(qn*q) for the positive scalar c).
  - k's rmsnorm scale c_n is applied to v instead of k (kv and ksum are
    bilinear): k_sb = relu(k'), v_sb = c'*v'.  Stats come from an ACT
    Square+accum pass (no DVE bn_stats).
  - W_{q,k,v} are scaled x32 on the host so the fp8 split is in e4m3's
    sweet spot; W_out x32 likewise.  Scale bookkeeping (values on the
    graded path):
      k' = 32k, c' = c/32, v_sb = c'*(32v) = cv, k_sb = relu(32k)
      kv block = 32*kv_norm, ksum col (c-col = c') = ksum_norm
      qT = 32*relu(q), denomT = 32*denom, rec = 8/(denomT + 32eps)
      attn8 = (1024*attn)*rec = 256*attn_norm  (fp8-friendly range)
      out_ps = attn8 @ (32*W_out) = 8192*out  -> final copy scale 2^-13.

All projections (qkv both orientations, and the output projection) run
as compensated-fp8 DoubleRow matmuls: a ~ a8+alo8 (e4m3), computing
a8b8 + a8blo8 + alo8b8.  DoubleRow packs 2 contraction k-tiles per
instruction at 0.5 cyc/row, so the 3-term scheme costs 0.75x of bf16
with ~1.3e-3 error.  x and W splits happen on the host; attnT is split
on-device (ACT fp8 copy + DVE subtract).

Per core, per group (1024 tokens = 8 token-tiles of 128):
  phase 1 per tile: k, v chunks (normal orientation, fp8-DR), ACT
    epilogues, kv_ext += k_sb^T v_ext in a PSUM-resident [128, 8, 129]
    accumulator (bf16, k_sum fused as column 128).  qT = Wq^T x^T
    computed directly transposed (fp8-DR), ACT Relu copy to bf16.
  AllReduce(kv_ext) over the 4 cores of the batch; phase-2 prep (bd
    blockdiag + zero-padded ksum) runs on the idle gpsimd engine right
    after the collective, overlapping the other group's phase 1.
  phase 2: per-head normalizers denomT[16, tok] via padded-ksum lhsT,
    exact reciprocal, broadcast via selector matmul, attnT normalized
    on the DVE + split to fp8, out = attnT^T @ W_out (fp8-DR).
"""

import os
import sys

import numpy as np
import ml_dtypes

for _p in ("/opt/trn_rl_repo",):
    if _p not in sys.path and os.path.isdir(_p):
        sys.path.insert(0, _p)

import concourse.mybir as mybir
import concourse.tile as tile
from concourse import bacc
from concourse.bass_utils import run_bass_kernel_spmd
from contextlib import ExitStack

F32 = mybir.dt.float32
F32R = mybir.dt.float32r
BF16 = mybir.dt.bfloat16
F8 = mybir.dt.float8e4
ALU = mybir.AluOpType
ACTF = mybir.ActivationFunctionType
DR = mybir.MatmulPerfMode.DoubleRow

DIM = 1024
HEADS = 16
DHEAD = 64
NPAIR = HEADS // 2          # 8 head pairs
B = 4
N = 4096
TOK = 2048                  # tokens per core (2 groups x 1024)
GTOK = 1024                 # tokens per group
NTG = GTOK // 128           # 8 token tiles per group
WSCALE = 32.0               # host premultiplier on W_{q,k,v} and W_out
EPS_NORM = 1e-6
EPS_DEN = 1e-6
KVW = 129                   # kv width per pair: 2*64 + ksum column
RBOOST = float(2 ** 20)     # reciprocal boost into fp16 normal range
RG = [[0, 1, 2, 3], [4, 5, 6, 7]]

NPF8 = ml_dtypes.float8_e4m3
NPBF = ml_dtypes.bfloat16

_CACHE: dict = {}


def _build(use_bias: bool, use_w: bool, sim_mode: bool = False):
    ndev = 1 if sim_mode else 8
    nc = bacc.Bacc("TRN2", target_bir_lowering=False, debug=False, num_devices=ndev)

    x8_d = nc.dram_tensor("x8", [128, 8, TOK], F8, kind="ExternalInput").ap()
    xr8_d = nc.dram_tensor("xr8", [128, 8, TOK], F8, kind="ExternalInput").ap()
    w8_d = nc.dram_tensor("w8", [128, 8, 3 * DIM], F8, kind="ExternalInput").ap()
    r8_d = nc.dram_tensor("r8", [128, 8, 3 * DIM], F8, kind="ExternalInput").ap()
    wout_d = nc.dram_tensor("wout", [8, 128, DIM], BF16, kind="ExternalInput").ap()
    bpat_d = nc.dram_tensor("bpat", [16, NPAIR, 128], F32R,
                            kind="ExternalInput").ap()
    if use_w:
        kn_d = nc.dram_tensor("kn", [128, DIM], F32, kind="ExternalInput").ap()
    if use_bias:
        bout_d = nc.dram_tensor("bout", [128, DIM], F32, kind="ExternalInput").ap()
    out_d = nc.dram_tensor("out", [TOK, DIM], BF16, kind="ExternalOutput").ap()
    debug = bool(os.environ.get("K2_DEBUG"))
    if debug:
        dbg_kvf = nc.dram_tensor("dbg_kvf", [128, NPAIR, KVW], F32,
                                 kind="ExternalOutput").ap()
        dbg_rec = nc.dram_tensor("dbg_rec", [16, 512], F32R,
                                 kind="ExternalOutput").ap()
        dbg_atn = nc.dram_tensor("dbg_atn", [128, NPAIR, 512], BF16,
                                 kind="ExternalOutput").ap()
        dbg_qt = nc.dram_tensor("dbg_qt", [128, GTOK], BF16,
                                kind="ExternalOutput").ap()

    with tile.TileContext(nc) as tc:
        with ExitStack() as outer:
            const = outer.enter_context(tc.tile_pool(name="const", bufs=1))
            wpool = outer.enter_context(tc.tile_pool(name="wpool", bufs=1))
            qTpool = outer.enter_context(tc.tile_pool(name="qTpool", bufs=2))
            xpool = outer.enter_context(tc.tile_pool(name="xpool", bufs=2))
            prep = outer.enter_context(tc.tile_pool(name="prep", bufs=2))
            drampool = outer.enter_context(
                tc.tile_pool(name="dram", bufs=1, space="DRAM")
            )

            ones8 = const.tile([128, 8], F32, name="ones8")
            nc.vector.memset(ones8[:], 1.0)
            epsb = const.tile([128, 1], F32, name="epsb")
            nc.vector.memset(epsb[:], 1024.0 * EPS_NORM)
            # selector for broadcasting recT rows to 64-row blocks:
            # bpat[2j+par, j, m] = 1 iff (m < 64) == (par == 0)
            bpat = const.tile([16, NPAIR, 128], F32R, name="bpat")
            nc.sync.dma_start(bpat[:], bpat_d[:])
            if use_w:
                kn_sb = const.tile([128, DIM], F32, name="kn_sb")
                nc.sync.dma_start(kn_sb[:], kn_d[:])
            if use_bias:
                bout_sb = const.tile([128, DIM], F32, name="bout_sb")
                nc.sync.dma_start(bout_sb[:], bout_d[:])

            # W_qkv fp8 pair, resident, streamed in first-use order;
            # the first k chunk is fine-grained so tile 0 starts early.
            w8_sb = wpool.tile([128, 8, 3 * DIM], F8, name="w8_sb")
            r8_sb = wpool.tile([128, 8, 3 * DIM], F8, name="r8_sb")
            for cp in range(4):
                nc.sync.dma_start(
                    w8_sb[:, 2 * cp : 2 * cp + 2, 1024:1536],
                    w8_d[:, 2 * cp : 2 * cp + 2, 1024:1536],
                )
            nc.sync.dma_start(r8_sb[:, :, 1024:1536], r8_d[:, :, 1024:1536])
            for f in (3, 4, 5, 0, 1):
                cs = slice(f * 512, (f + 1) * 512)
                nc.sync.dma_start(w8_sb[:, :, cs], w8_d[:, :, cs])
                nc.sync.dma_start(r8_sb[:, :, cs], r8_d[:, :, cs])
            wout_sb = []
            for c in range(8):
                w = wpool.tile([128, DIM], BF16, name=f"wo{c}")
                nc.sync.dma_start(w[:], wout_d[c, :, :])
                wout_sb.append(w)

            # x fp8 pairs for both groups, prefetched on the Pool queue.
            x8g, xr8g = {}, {}
            for g in range(2):
                x8g[g] = xpool.tile([128, 8, GTOK], F8, name=f"x8g{g}", tag="x8g")
                xr8g[g] = xpool.tile([128, 8, GTOK], F8, name=f"xr8g{g}",
                                     tag="xr8g")
            for g in range(2):
                for t in range(NTG):
                    ts = slice(t * 128, (t + 1) * 128)
                    gts = slice(g * GTOK + t * 128, g * GTOK + (t + 1) * 128)
                    # warmup: first tile rides the idle ACT queue so its
                    # dispatch overlaps SP's weight streaming
                    eng = nc.scalar if (g == 0 and t == 0) else nc.gpsimd
                    eng.dma_start(x8g[g][:, :, ts], x8_d[:, :, gts])
                    eng.dma_start(xr8g[g][:, :, ts], xr8_d[:, :, gts])

            qT = {}
            bd_t, ksp_t = {}, {}

            with ExitStack() as ph1:
                qkp = ph1.enter_context(tc.tile_pool(name="qkp", bufs=4))
                vp = ph1.enter_context(tc.tile_pool(name="vp", bufs=4))
                sqp = ph1.enter_context(tc.tile_pool(name="sqp", bufs=2))
                stats = ph1.enter_context(tc.tile_pool(name="stats", bufs=4))
                psproj = ph1.enter_context(
                    tc.tile_pool(name="psproj", bufs=5, space="PSUM")
                )
                pskv = ph1.enter_context(
                    tc.tile_pool(name="pskv", bufs=1, space="PSUM")
                )
                arp = ph1.enter_context(tc.tile_pool(name="arp", bufs=2))

                def dr_chain(ps, lhs_pair, rhs_pair):
                    """3-term compensated fp8 DoubleRow accumulation chain,
                    hi*hi terms first so the chain can start before the lo
                    tensors arrive."""
                    steps = (
                        [(lhs_pair(0, cp), rhs_pair(0, cp)) for cp in range(4)]
                        + [(lhs_pair(0, cp), rhs_pair(1, cp)) for cp in range(4)]
                        + [(lhs_pair(1, cp), rhs_pair(0, cp)) for cp in range(4)]
                    )
                    for i, (l, r) in enumerate(steps):
                        nc.tensor.matmul(
                            ps[:], l, r,
                            start=(i == 0), stop=(i == len(steps) - 1),
                            perf_mode=DR,
                        )

                for g in range(2):
                    # PSUM allows one accumulation group per 2KB bank: kv
                    # packs 4 pairs per bank (start only on the bank's first
                    # write, stop on its last); ksum gets its own bank.
                    kvps = pskv.tile([128, NPAIR, 128], F32, name=f"kvps{g}",
                                     tag="kv")
                    ksps = pskv.tile([128, NPAIR], F32, name=f"ksps{g}",
                                     tag="ks")
                    qT[g] = [
                        qTpool.tile([128, GTOK], BF16, name=f"qT{g}_{j}",
                                    tag=f"qT{j}")
                        for j in range(NPAIR)
                    ]
                    x8, xr8 = x8g[g], xr8g[g]

                    pend = []  # lagged kv matmuls: (tile, k_sb, vext, c_bf)

                    def flush_kv():
                        while pend:
                            t, k_sb, vext, c_bf = pend.pop(0)
                            for p in range(NPAIR):
                                nc.tensor.matmul(
                                    kvps[:, p, :],
                                    k_sb[:, p * 128 : (p + 1) * 128],
                                    vext[:, p, :],
                                    start=(t == 0 and p % 4 == 0),
                                    stop=(t == NTG - 1 and p % 4 == 3),
                                    skip_group_check=True,
                                )
                            for p in range(NPAIR):
                                nc.tensor.matmul(
                                    ksps[:, p : p + 1],
                                    k_sb[:, p * 128 : (p + 1) * 128],
                                    c_bf[:],
                                    start=(t == 0 and p == 0),
                                    stop=(t == NTG - 1 and p == NPAIR - 1),
                                    skip_group_check=True,
                                )

                    def qT_chunk(tc_i):
                        tsl = slice(tc_i * 512, (tc_i + 1) * 512)
                        for j in range(NPAIR):
                            qps = psproj.tile([128, 512], F32, name=f"q{g}{tc_i}{j}",
                                              tag="pp")
                            dr_chain(
                                qps,
                                lambda hi, cp, j=j: (w8_sb if hi == 0 else r8_sb)[
                                    :, 2 * cp : 2 * cp + 2, j * 128 : (j + 1) * 128
                                ],
                                lambda hi, cp, tsl=tsl: (x8 if hi == 0 else xr8)[
                                    :, 2 * cp : 2 * cp + 2, tsl
                                ],
                            )
                            nc.scalar.activation(qT[g][j][:, tsl], qps[:], ACTF.Relu)

                    def proj_chunks(tiles, fs):
                        """chunk-major DR chains over a tile pair: keeps PE
                        fed from the already-arrived weight slice while the
                        next one streams."""
                        tab = {}
                        for f in fs:
                            for t in tiles:
                                ts = slice(t * 128, (t + 1) * 128)
                                ps = psproj.tile([128, 512], F32,
                                                 name=f"p{g}{t}{f}", tag="pp")
                                dr_chain(
                                    ps,
                                    lambda hi, cp, ts=ts: (x8 if hi == 0 else xr8)[
                                        :, 2 * cp : 2 * cp + 2, ts
                                    ],
                                    lambda hi, cp, f=f: (
                                        w8_sb if hi == 0 else r8_sb
                                    )[:, 2 * cp : 2 * cp + 2,
                                      f * 512 : (f + 1) * 512],
                                )
                                tab[(t, f)] = ps
                        return tab

                    def k_epilogue(t, pk):
                        sq = sqp.tile([128, 512], BF16, name=f"sq{g}{t}", tag="sq")
                        acc = stats.tile([128, 2], F32, name=f"acc{g}{t}", tag="acc")
                        nc.scalar.activation(
                            sq[:], pk[0][:], ACTF.Square, accum_out=acc[:, 0:1]
                        )
                        nc.scalar.activation(
                            sq[:], pk[1][:], ACTF.Square, accum_out=acc[:, 1:2]
                        )
                        ssum = stats.tile([128, 1], F32, name=f"ss{g}{t}", tag="ss")
                        nc.vector.tensor_tensor(
                            ssum[:], acc[:, 0:1], acc[:, 1:2], ALU.add
                        )
                        srt = stats.tile([128, 1], F32, name=f"sr{g}{t}", tag="sr")
                        # c' = 1/sqrt(ssum/1024 + 1024*eps) = c_true/32
                        nc.scalar.activation(
                            srt[:], ssum[:], ACTF.Sqrt,
                            scale=1.0 / 1024.0, bias=epsb[:],
                        )
                        c_t = stats.tile([128, 1], F32, name=f"c{g}{t}", tag="c")
                        nc.vector.reciprocal(c_t[:], srt[:])
                        c_bf = stats.tile([128, 1], BF16, name=f"cb{g}{t}",
                                          tag="cb")
                        nc.vector.tensor_copy(c_bf[:], c_t[:])

                        k_sb = qkp.tile([128, DIM], BF16, name=f"ksb{g}{t}",
                                        tag="ksb")
                        if use_w:
                            # kn_w path: k_sb = relu(kn * k')
                            kwt = sqp.tile([128, 512], F32, name=f"kw{g}{t}",
                                           tag="kw")
                            for h in range(2):
                                fs = slice(h * 512, (h + 1) * 512)
                                nc.vector.tensor_tensor(
                                    kwt[:], pk[h][:],
                                    kn_sb[:, h * 512 : (h + 1) * 512],
                                    ALU.mult,
                                )
                                nc.vector.tensor_scalar_max(k_sb[:, fs], kwt[:],
                                                            0.0)
                        else:
                            for h in range(2):
                                fs = slice(h * 512, (h + 1) * 512)
                                nc.scalar.activation(k_sb[:, fs], pk[h][:],
                                                     ACTF.Relu)
                        return k_sb, c_t, c_bf

                    def v_epilogue(t, pv, c_t):
                        vext = vp.tile([128, NPAIR, 128], BF16, name=f"ve{g}{t}",
                                       tag="ve")
                        for h in range(2):
                            nc.scalar.activation(
                                vext[:, 4 * h : 4 * h + 4, :],
                                pv[h][:].rearrange("p (a b) -> p a b", b=128),
                                ACTF.Copy,
                                scale=c_t[:],
                            )
                        return vext

                    for tp in range(NTG // 2):
                        tiles = (2 * tp, 2 * tp + 1)
                        ktab = proj_chunks(tiles, (2, 3))
                        # lagged kv for the previous pair while ACT drains k
                        flush_kv()
                        keps = {t: k_epilogue(t, [ktab[(t, 2)], ktab[(t, 3)]])
                                for t in tiles}
                        vtab = proj_chunks(tiles, (4, 5))
                        for t in tiles:
                            k_sb, c_t, c_bf = keps[t]
                            vext = v_epilogue(t, [vtab[(t, 4)], vtab[(t, 5)]], c_t)
                            pend.append((t, k_sb, vext, c_bf))
                        if tp == 1:
                            qT_chunk(0)
                    flush_kv()

                    # ---- stage AllReduce (before the last qT chunk so the
                    # collective overlaps its ~10us of PE work) ----
                    kvf = arp.tile([128, NPAIR, KVW], F32, name=f"kvf{g}", tag="kvf")
                    nc.scalar.copy(
                        kvf[:, 0:4, 0:128], kvps[:, 0:4, :]
                    )
                    nc.scalar.copy(
                        kvf[:, 4:8, 0:128], kvps[:, 4:8, :]
                    )
                    nc.scalar.copy(kvf[:, :, 128], ksps[:])
                    arin = drampool.tile(
                        [128, NPAIR, KVW], F32, name=f"arin{g}", tag=f"arin{g}"
                    )
                    nc.sync.dma_start(arin[:], kvf[:])
                    arout = drampool.tile(
                        [128, NPAIR, KVW], F32, name=f"arout{g}", tag=f"arout{g}"
                    )
                    if sim_mode:
                        nc.gpsimd.dma_start(arout[:], arin[:])
                    else:
                        nc.gpsimd.collective_compute(
                            "AllReduce",
                            ALU.add,
                            replica_groups=RG,
                            ins=[arin.opt()],
                            outs=[arout.opt()],
                        )

                    # ---- phase-2 prep on the idle gpsimd engine ----
                    kvr = prep.tile([128, NPAIR, KVW], F32, name=f"kvr{g}",
                                    tag="kvr")
                    nc.gpsimd.dma_start(kvr[:], arout[:])
                    if debug and g == 0:
                        nc.gpsimd.dma_start(dbg_kvf[:], kvr[:])
                        nc.gpsimd.dma_start(dbg_qt[:], qT[0][0][:])
                    bd = prep.tile([128, NPAIR, 128], BF16, name=f"bd{g}", tag="bd")
                    nc.gpsimd.memset(bd[:].rearrange("p a b -> p (a b)"), 0.0)
                    ksp = prep.tile([128, NPAIR, 16], BF16, name=f"ksp{g}",
                                    tag="ksp")
                    nc.gpsimd.memset(ksp[:].rearrange("p a b -> p (a b)"), 0.0)
                    for j in range(NPAIR):
                        nc.gpsimd.tensor_copy(bd[0:64, j, 0:64], kvr[0:64, j, 0:64])
                        nc.gpsimd.tensor_copy(
                            bd[64:128, j, 64:128], kvr[64:128, j, 64:128]
                        )
                        nc.gpsimd.tensor_copy(
                            ksp[0:64, j, 2 * j : 2 * j + 1], kvr[0:64, j, 128:129]
                        )
                        nc.gpsimd.tensor_copy(
                            ksp[64:128, j, 2 * j + 1 : 2 * j + 2],
                            kvr[64:128, j, 128:129],
                        )
                    bd_t[g], ksp_t[g] = bd, ksp

                    qT_chunk(1)

            # ------------- phase 2 -------------
            with ExitStack() as ph2:
                atp = ph2.enter_context(tc.tile_pool(name="atp", bufs=3))
                recp = ph2.enter_context(tc.tile_pool(name="recp", bufs=2))
                osbp = ph2.enter_context(tc.tile_pool(name="osbp", bufs=3))
                psden = ph2.enter_context(
                    tc.tile_pool(name="psden", bufs=2, space="PSUM")
                )
                psat = ph2.enter_context(
                    tc.tile_pool(name="psat", bufs=2, space="PSUM")
                )
                psout = ph2.enter_context(
                    tc.tile_pool(name="psout", bufs=2, space="PSUM")
                )

                atall = {}

                def split_block(g, tc_i, ointer=None):
                    """denominators -> reciprocal -> normalized attnT (bf16).
                    rb staged to SBUF on ACT (DVE reads one PSUM max), the
                    normalize mult on DVE.  `ointer`: generator yielding
                    out-proj instruction bundles, interleaved per j to keep
                    the PE busy while the conveyor drains."""
                    bd, ksp = bd_t[g], ksp_t[g]
                    tsl = slice(tc_i * 512, (tc_i + 1) * 512)
                    dps = psden.tile([16, 512], F32, name=f"d{g}{tc_i}",
                                     tag="dps")
                    for j in range(NPAIR):
                        nc.tensor.matmul(
                            dps[:],
                            ksp[:, j, :],
                            qT[g][j][:, tsl],
                            start=(j == 0),
                            stop=(j == NPAIR - 1),
                        )
                    rec = recp.tile([16, 512], F32R, name=f"r{g}{tc_i}",
                                    tag="rec")
                    # rec = RBOOST/(dps + 32*eps): keeps the fp16-staged
                    # reciprocal in normal range (raw values ~1e-6 would be
                    # fp16 subnormals); 1/(32*RBOOST) folds into the output.
                    nc.vector.tensor_scalar(
                        rec[:], dps[:], 1.0 / RBOOST,
                        WSCALE * EPS_DEN / RBOOST, ALU.mult, ALU.add,
                    )
                    with nc.allow_low_precision(reason="f32r is fp32"):
                        nc.vector.reciprocal(rec[:], rec[:])
                    if debug and g == 0 and tc_i == 0:
                        nc.sync.dma_start(dbg_rec[:], rec[:])

                    atn = atp.tile([128, NPAIR, 512], BF16, name=f"at{g}{tc_i}",
                                   tag="atn")
                    for j in range(NPAIR):
                        rb = psat.tile([128, 512], F32, name=f"rb{g}{tc_i}{j}",
                                       tag="rb")
                        nc.tensor.matmul(rb[:], bpat[:, j, :], rec[:])
                        rbc = recp.tile([128, 512], mybir.dt.float16,
                                        name=f"rc{g}{tc_i}{j}",
                                        tag=f"rbc{j % 2}")
                        nc.scalar.copy(rbc[:], rb[:])
                        at = psat.tile([128, 512], F32, name=f"a{g}{tc_i}{j}",
                                       tag="at")
                        nc.tensor.matmul(at[:], bd[:, j, :], qT[g][j][:, tsl])
                        nc.vector.tensor_tensor(atn[:, j, :], at[:], rbc[:],
                                                ALU.mult)
                        if ointer is not None:
                            next(ointer, None)
                    if debug and g == 0 and tc_i == 0:
                        nc.sync.dma_start(dbg_atn[:], atn[:])
                    atall[(g, tc_i)] = atn

                def out_block(g, tc_i):
                    """generator: yields after every (tt, ff) half so it can
                    be interleaved into the next split block."""
                    atn = atall[(g, tc_i)]
                    for tt in range(4):
                        r0 = g * GTOK + tc_i * 512 + tt * 128
                        tts = slice(tt * 128, (tt + 1) * 128)
                        ops = [
                            psout.tile([128, 512], F32,
                                       name=f"o{g}{tc_i}{tt}{ff}", tag="ops")
                            for ff in range(2)
                        ]
                        for ff in range(2):
                            for j in range(NPAIR):
                                nc.tensor.matmul(
                                    ops[ff][:],
                                    atn[:, j, tts],
                                    wout_sb[j][:, ff * 512 : (ff + 1) * 512],
                                    start=(j == 0),
                                    stop=(j == NPAIR - 1),
                                )
                            yield
                        for ff in range(2):
                            fsl = slice(ff * 512, (ff + 1) * 512)
                            osb = osbp.tile([128, 512], BF16,
                                            name=f"os{g}{tc_i}{tt}{ff}",
                                            tag="osb")
                            if use_bias:
                                nc.vector.scalar_tensor_tensor(
                                    out=osb[:], in0=ops[ff][:],
                                    scalar=1.0 / (WSCALE * RBOOST),
                                    in1=bout_sb[:, fsl],
                                    op0=ALU.mult, op1=ALU.add,
                                )
                            else:
                                nc.scalar.activation(
                                    osb[:], ops[ff][:], ACTF.Copy,
                                    scale=1.0 / (WSCALE * RBOOST),
                                )
                            nc.sync.dma_start(
                                out_d[r0 : r0 + 128, fsl], osb[:]
                            )

                def drain(gen):
                    if gen is not None:
                        for _ in gen:
                            pass

                # block-level software pipeline: split(i) interleaves the
                # out-projection of block i-1 so the PE stays fed while the
                # split conveyor (ACT/DVE/Pool) drains.
                blocks = [(0, 0), (0, 1), (1, 0), (1, 1)]
                split_block(*blocks[0])
                split_block(*blocks[1])
                prev = out_block(*blocks[0])
                split_block(*blocks[2], ointer=prev)
                drain(prev)
                prev = out_block(*blocks[1])
                split_block(*blocks[3], ointer=prev)
                drain(prev)
                drain(out_block(*blocks[2]))
                drain(out_block(*blocks[3]))

    nc.compile()
    return nc


def _get_nc(use_bias: bool, use_w: bool):
    key = ("nc", use_bias, use_w)
    if key not in _CACHE:
        _CACHE[key] = _build(use_bias, use_w)
    return _CACHE[key]


def _f8split(a):
    hi = a.astype(NPF8)
    lo = (a - hi.astype(np.float32)).astype(NPF8)
    return hi, lo


def make_in_maps(x, W_qkv, qn_w, kn_w, W_out, b_out):
    x = np.asarray(x, dtype=np.float32)
    W = np.array(np.asarray(W_qkv, dtype=np.float32))
    qn = np.asarray(qn_w, dtype=np.float32)
    kn = np.asarray(kn_w, dtype=np.float32)
    use_w = not np.all(kn == 1.0)
    W[:, :DIM] = W[:, :DIM] * qn[None, :]
    W8, R8 = _f8split(np.ascontiguousarray(
        (W * WSCALE).reshape(8, 128, 3 * DIM).transpose(1, 0, 2)
    ))
    wout = np.ascontiguousarray(
        np.asarray(W_out, dtype=np.float32).reshape(8, 128, DIM)
    ).astype(NPBF)
    kn_b = np.ascontiguousarray(np.broadcast_to(kn.reshape(1, DIM), (128, DIM)))
    bout = np.ascontiguousarray(
        np.broadcast_to(np.asarray(b_out, dtype=np.float32).reshape(1, DIM),
                        (128, DIM))
    )
    x8f, xr8f = _f8split(x)  # [4, 4096, 1024]
    bpat_h = np.zeros((16, NPAIR, 128), dtype=np.float32)
    for j in range(NPAIR):
        bpat_h[2 * j, j, 0:64] = 1.0
        bpat_h[2 * j + 1, j, 64:128] = 1.0
    in_maps = []
    for c in range(8):
        b0 = 2 * (c // 4)
        q = c % 4
        sl = slice(1024 * q, 1024 * (q + 1))

        def arrange(xs):
            gs = []
            for b in (b0, b0 + 1):
                xt = xs[b, sl, :].T  # [1024 dim, 1024 tok]
                gs.append(xt.reshape(8, 128, GTOK).transpose(1, 0, 2))
            return np.ascontiguousarray(np.concatenate(gs, axis=2))

        m = {
            "x8": arrange(x8f),
            "xr8": arrange(xr8f),
            "w8": np.ascontiguousarray(W8),
            "r8": np.ascontiguousarray(R8),
            "wout": wout,
            "bpat": bpat_h,
        }
        if use_w:
            m["kn"] = kn_b
        if np.any(bout):
            m["bout"] = bout
        in_maps.append(m)
    return in_maps


def assemble(results):
    out = np.empty((B, N, DIM), dtype=np.float32)
    for b in range(B):
        base = 4 * (b // 2)
        g = b % 2
        for q in range(4):
            out[b, 1024 * q : 1024 * (q + 1), :] = results[base + q]["out"][
                1024 * g : 1024 * (g + 1), :
            ].astype(np.float32)
    return out


def run(in_maps, use_bias, use_w, **kw):
    nc = _get_nc(use_bias, use_w)
    return run_bass_kernel_spmd(nc, in_maps, core_ids=list(range(8)), **kw)


def kernel(x, W_qkv, qn_w, kn_w, W_out, b_out):
    use_bias = bool(np.any(np.asarray(b_out)))
    use_w = not np.all(np.asarray(kn_w) == 1.0)
    in_maps = make_in_maps(x, W_qkv, qn_w, kn_w, W_out, b_out)
    res = run(in_maps, use_bias, use_w)
    return assemble(res.results)
